# revision 16
# baseline (speedup 1.0000x reference)
"""CycleMLP 1w1a (binary cycle-shift conv + 1x1 GEMM) for 8 Trainium2 cores.

  out[b,o,h,w] = sum_c sign(weight)[o,c] * sign(x)[b,c,h,w+off(c)] + bias[o]
  off(c) = (c+3) % 7 - 3, zero-padded outside [0, W)

Sharding: data-parallel over batch B=64 -> 8 batches/core; weight/bias
replicated.

Key layout tricks (all host-side, mathematically identity):
  - channels permuted so shift-groups (residue c % 7) are contiguous and
    ordered by DESCENDING shift d; the weight's contraction dim is permuted
    identically.  With one pad element inserted between groups in the DRAM
    buffer, consecutive shifted per-channel windows tile the buffer exactly
    contiguously, so each 128-channel chunk loads as ONE dense full-partition
    2D DMA (the only transfer shape that fans evenly across all 16 SDMA
    engines; sub-128-partition or lattice transfers pile onto engine 0).
    The shift itself is still performed by the device DMA via the base
    offset; all group/row-boundary leaks land in masked columns.
  - per-core layout [G, C*BG*HW] (channel-major per half-batch group):
    6 input loads + 6 stores of ~1 MB each per core, total.
  - fp8e5m2 x (sign-preserving except |x| < 2^-17, P ~ 3e-6) and bf16 out
    (integer sums <= 384 round with ulp 0.125 -> rel err ~5e-4) cut HBM
    traffic to 0.25x + 0.5x of f32.

Per-core kernel, phase-ordered so no engine stream blocks another:
  all loads (Sync HWDGE) | sign per chunk (ScalarE) | boundary mask-mul
  (GpSimd) | GEMM 3m x 8n x 3k with PSUM K-accum (TensorE) | bias-add
  eviction to bf16 (DVE) | stores (Sync HWDGE, queued after all loads).
"""

import sys

for p in ("/opt/trn_rl_repo", "/root/.axon_site/_ro/trn_rl_repo"):
    if p not in sys.path:
        sys.path.append(p)

import numpy as np

B = 64
C = 384
H = W = 32
HW = H * W
KW = 7
NK = 3  # contraction chunks of 128
NM = 3  # output-channel chunks of 128
NTILE = 512
N_CORES = 8
SB = B // N_CORES  # batches per core
G = 4  # pipeline groups per core
BG = SB // G  # batches per group
FREE = BG * HW  # free dim per tile
LEAD = 3  # read-base offset = d of the first (largest-d) group
PADLEN = C * FREE + 2 * LEAD  # + one pad elem between the 7 groups

_CACHE = {}

# residues ordered by descending shift d = (r+3)%7-3
_RES_ORDER = sorted(range(KW), key=lambda r: -((r + 3) % KW - KW // 2))


def _perm_and_segs():
    """Channel permutation (residue groups, descending d) and per-chunk
    segments segs[k] = [(p0, p1, d)] (partitions [128k+p0, 128k+p1))."""
    perm = []
    segs = [[] for _ in range(NK)]
    i = 0
    for r in _RES_ORDER:
        chans = [c for c in range(C) if c % KW == r]
        perm.extend(chans)
        d = (r + 3) % KW - KW // 2
        lo, hi = i, i + len(chans)
        while lo < hi:
            k = lo // 128
            seg_hi = min(hi, (k + 1) * 128)
            segs[k].append((lo - 128 * k, seg_hi - 128 * k, d))
            lo = seg_hi
        i = hi
    return np.asarray(perm), segs


_PERM, _SEGS = _perm_and_segs()
# group boundaries in permuted index space (for pad insertion)
_GROUP_SIZES = [len([c for c in range(C) if c % KW == r]) for r in _RES_ORDER]


def _prep_weights(weight, bias):
    import ml_dtypes

    # weights scaled to +-2: the device computes g = (x >= 0) - 0.5 in {+-0.5},
    # so products are +-1 and PSUM sums match the +-1 x +-1 reference exactly
    wb = np.sign(weight.astype(np.float32)) * 2.0  # [O, C]
    lhsT = np.ascontiguousarray(wb.T[_PERM, :])  # [C_perm, O]
    wt = np.ascontiguousarray(lhsT.reshape(NK, 128, C).transpose(1, 0, 2)).astype(
        ml_dtypes.float8_e5m2
    )  # [128, NK, C]
    bias_sb = np.ascontiguousarray(bias.astype(np.float32).reshape(NM, 128).T)

    mask = np.ones((128, NK, W), dtype=np.float32)
    for k in range(NK):
        for (p0, p1, d) in _SEGS[k]:
            if d > 0:
                mask[p0:p1, k, W - d : W] = 0.0
            elif d < 0:
                mask[p0:p1, k, 0:-d] = 0.0
    mask = mask.astype(ml_dtypes.float8_e5m2)
    return wt, bias_sb, mask


def _legalize_waits(nc, max_waits=1):
    """Walrus for this toolchain accepts at most one sem wait per
    instruction.  Split instructions carrying more into preceding
    same-engine NoOps (engine streams are in-order, so the split is
    semantically identical to the combined wait)."""
    import concourse.mybir as mybir

    fn = nc.m.functions[0]
    ctr = 0
    for blk in fn.blocks:
        out = []
        changed = False
        for inst in blk.instructions:
            si = inst.sync_info
            waits = list(si.on_wait) if si is not None and si.on_wait else []
            if len(waits) > max_waits and str(inst.engine) != "EngineType.Unassigned":
                keep = waits[-max_waits:]
                extra = waits[:-max_waits]
                for j in range(0, len(extra), max_waits):
                    nop = mybir.InstNoOp(name=f"I-waitsplit-{ctr}")
                    ctr += 1
                    nop.engine = inst.engine
                    nop.sync_info = mybir.SyncInfo(
                        on_wait=extra[j : j + max_waits], on_update=[]
                    )
                    out.append(nop)
                si.on_wait = keep
                changed = True
            out.append(inst)
        if changed:
            blk.instructions = out
    return ctr


def _build(psum_bufs=8, ost_bufs=4, legalize=True):
    import concourse.bass as bass
    import concourse.mybir as mybir
    import concourse.tile as tile
    from concourse.ap import AP

    nc = bass.Bass()
    x_d = nc.declare_dram_parameter("x", [G, PADLEN], mybir.dt.float8e5, isOutput=False)
    wt_d = nc.declare_dram_parameter("wt", [128, NK, C], mybir.dt.float8e5, isOutput=False)
    bias_d = nc.declare_dram_parameter("bias", [128, NM], mybir.dt.float32, isOutput=False)
    mask_d = nc.declare_dram_parameter("mask", [128, NK, W], mybir.dt.float8e5, isOutput=False)
    out_d = nc.declare_dram_parameter("out", [G, C, FREE], mybir.dt.bfloat16, isOutput=True)

    with tile.TileContext(nc) as tc:
        with (
            tc.tile_pool(name="const", bufs=1) as const_pool,
            tc.tile_pool(name="raw", bufs=G * NK) as raw_pool,
            tc.tile_pool(name="g", bufs=G) as g_pool,
            tc.tile_pool(name="ost", bufs=ost_bufs) as ost_pool,
            tc.tile_pool(name="ps", bufs=psum_bufs, space="PSUM") as ps_pool,
        ):
            wt = const_pool.tile([128, NK, C], mybir.dt.float8e5)
            bias_sb = const_pool.tile([128, NM], mybir.dt.float32)
            mask_sb = const_pool.tile([128, NK, W], mybir.dt.float8e5)
            nc.sync.dma_start(wt[:], wt_d[:])
            nc.sync.dma_start(bias_sb[:], bias_d[:])
            nc.sync.dma_start(mask_sb[:], mask_d[:])

            # phase L: all input loads up front (dense full-partition 2D)
            raws = {}
            for g in range(G):
                for k in range(NK):
                    raw = raw_pool.tile([128, FREE], mybir.dt.float8e5, tag="raw")
                    src = AP(
                        tensor=x_d,
                        offset=g * PADLEN + LEAD + k * 128 * FREE,
                        ap=[[FREE, 128], [1, FREE]],
                    )
                    nc.sync.dma_start(raw[:], src)
                    raws[(g, k)] = raw

            # phase C: binarize g = (x >= 0) - 0.5 in {+-0.5} (one fused
            # two-op tensor_scalar on DVE) + boundary mask multiply (DVE)
            g01s = {}
            g2s = {}
            for g in range(G):
                g01 = g_pool.tile([128, 2, FREE], mybir.dt.float8e5, tag="g01")
                g2 = g_pool.tile([128, FREE], mybir.dt.float8e5, tag="g2")
                for k in range(NK):
                    raw = raws[(g, k)]
                    gk = g01[:, k, :] if k < 2 else g2[:]
                    nc.vector.tensor_scalar(
                        gk,
                        raw[:],
                        0.0,
                        0.5,
                        mybir.AluOpType.is_ge,
                        mybir.AluOpType.subtract,
                    )
                    if k < 2:
                        v = g01.rearrange("p two (r w) -> p two r w", w=W)[
                            :, k, :, :
                        ]
                    else:
                        v = g2.rearrange("p (r w) -> p r w", w=W)
                    mk = mask_sb[:, k : k + 1, :].broadcast_to([128, BG * H, W])
                    nc.vector.tensor_mul(v, v, mk)
                g01s[g] = g01
                g2s[g] = g2

            # phase M: GEMM (k-outer so consecutive matmuls share weights) +
            # bias eviction on ScalarE + stores (Sync ring, emitted after
            # every load so they never block a load issue)
            NT = FREE // NTILE
            for g in range(G):
                for m in range(NM):
                    pss = []
                    for _n in range(NT):
                        ps = ps_pool.tile(
                            [128, NTILE], mybir.dt.float32, tag="ps", name=f"ps{_n}"
                        )
                        pss.append(ps)
                    for n in range(NT):
                        nc.tensor.matmul(
                            pss[n][:],
                            wt[:, 0:2, m * 128 : (m + 1) * 128],
                            g01s[g][:, :, n * NTILE : (n + 1) * NTILE],
                            start=True,
                            stop=False,
                            perf_mode=mybir.MatmulPerfMode.DoubleRow,
                        )
                    for n in range(NT):
                        nc.tensor.matmul(
                            pss[n][:],
                            wt[:, 2, m * 128 : (m + 1) * 128],
                            g2s[g][:, n * NTILE : (n + 1) * NTILE],
                            start=False,
                            stop=True,
                        )
                    ost = ost_pool.tile([128, FREE], mybir.dt.bfloat16, tag="ost")
                    for n in range(NT):
                        nc.scalar.activation(
                            ost[:, n * NTILE : (n + 1) * NTILE],
                            pss[n][:],
                            mybir.ActivationFunctionType.Identity,
                            bias=bias_sb[:, m : m + 1],
                        )
                    nc.sync.dma_start(
                        out_d[g, m * 128 : (m + 1) * 128, :], ost[:]
                    )
    if legalize:
        _legalize_waits(nc)
    return nc


def _ensure_ntff_hook():
    """Register the axon NTFF profiling hook if the image's antenv lacks it."""
    import types

    try:
        from antenv.axon_hooks import get_axon_ntff_profile_hook  # noqa: F401

        return
    except ImportError:
        pass
    hook = None
    try:
        from trn_agent_boot.trn_boot import _ntff_profile_via_ctypes

        hook = _ntff_profile_via_ctypes("/opt/axon/libaxon_pjrt.so")
    except Exception:
        pass
    mod = types.ModuleType("antenv.axon_hooks")
    mod._hook = hook
    mod.get_axon_ntff_profile_hook = lambda: mod._hook
    mod.set_axon_ntff_profile_hook = lambda h: setattr(mod, "_hook", h)
    sys.modules["antenv.axon_hooks"] = mod
    try:
        import antenv

        antenv.axon_hooks = mod
    except Exception:
        pass


def _pack_x(x):
    """[B, C, H, W] f32 -> per-core [G, PADLEN] bf16 buffers with the
    permuted channel-major layout and 1-elem pads between shift groups."""
    import ml_dtypes

    xq = np.asarray(x, dtype=np.float32).reshape(B, C, HW)[:, _PERM, :].astype(
        ml_dtypes.float8_e5m2
    )
    shards = []
    for i in range(N_CORES):
        buf = np.zeros((G, PADLEN), dtype=ml_dtypes.float8_e5m2)
        for g in range(G):
            src = xq[i * SB + g * BG : i * SB + (g + 1) * BG]  # [BG, C, HW]
            xt = np.ascontiguousarray(src.transpose(1, 0, 2)).reshape(C, FREE)
            pos = 0
            a = 0
            for n in _GROUP_SIZES:
                buf[g, pos : pos + n * FREE] = xt[a : a + n].reshape(-1)
                pos += n * FREE + 1
                a += n
        shards.append(buf)
    return shards


def run(x, weight, bias, trace=False):
    """Returns (out [B,C,H,W] f32, exec_time_ns or None)."""
    import concourse.bass_utils as bu
    from concourse.bass_utils import run_bass_kernel_spmd

    if trace:
        _ensure_ntff_hook()
        # zero-egress container: don't try to copy trace artifacts to a bucket
        bu.upload_artifacts = lambda tmpdir: tmpdir

    if "nc" not in _CACHE:
        _CACHE["nc"] = _build()
    nc = _CACHE["nc"]

    wt, bias_sb, mask = _prep_weights(weight, bias)
    shards = _pack_x(x)
    in_maps = [
        {"x": shards[i], "wt": wt, "bias": bias_sb, "mask": mask}
        for i in range(N_CORES)
    ]
    res = run_bass_kernel_spmd(
        nc, in_maps, core_ids=list(range(N_CORES)), trace=trace
    )
    outs = []
    for i in range(N_CORES):
        o = np.asarray(res.results[i]["out"]).reshape(G, C, BG, HW)
        outs.append(o.transpose(0, 2, 1, 3).reshape(SB, C, HW))
    out = np.concatenate(outs, axis=0).astype(np.float32)
    return out.reshape(B, C, H, W), res.exec_time_ns


def kernel(x, weight, bias):
    out, _ = run(x, weight, bias, trace=False)
    return out


# revision 18
# speedup vs baseline: 1.2348x; 1.2348x over previous
"""CycleMLP 1w1a (binary cycle-shift conv + 1x1 GEMM) for 8 Trainium2 cores.

  out[b,o,h,w] = sum_c sign(weight)[o,c] * sign(x)[b,c,h,w+off(c)] + bias[o]
  off(c) = (c+3) % 7 - 3, zero-padded outside [0, W)

Sharding: data-parallel over batch B=64 -> 8 batches/core; weight/bias
replicated.

Key layout tricks (all host-side, mathematically identity):
  - channels permuted so shift-groups (residue c % 7) are contiguous and
    ordered by DESCENDING shift d; the weight's contraction dim is permuted
    identically.  With one pad element inserted between groups in the DRAM
    buffer, consecutive shifted per-channel windows tile the buffer exactly
    contiguously, so each 128-channel chunk loads as ONE dense full-partition
    2D DMA (the only transfer shape that fans evenly across all 16 SDMA
    engines; sub-128-partition or lattice transfers pile onto engine 0).
    The shift itself is still performed by the device DMA via the base
    offset; all group/row-boundary leaks land in masked columns.
  - per-core layout [G, C*BG*HW] (channel-major per half-batch group):
    6 input loads + 6 stores of ~1 MB each per core, total.
  - fp8e5m2 x (sign-preserving except |x| < 2^-17, P ~ 3e-6) and bf16 out
    (integer sums <= 384 round with ulp 0.125 -> rel err ~5e-4) cut HBM
    traffic to 0.25x + 0.5x of f32.

Per-core kernel, phase-ordered so no engine stream blocks another:
  all loads (Sync HWDGE) | sign per chunk (ScalarE) | boundary mask-mul
  (GpSimd) | GEMM 3m x 8n x 3k with PSUM K-accum (TensorE) | bias-add
  eviction to bf16 (DVE) | stores (Sync HWDGE, queued after all loads).
"""

import sys

for p in ("/opt/trn_rl_repo", "/root/.axon_site/_ro/trn_rl_repo"):
    if p not in sys.path:
        sys.path.append(p)

import numpy as np

B = 64
C = 384
H = W = 32
HW = H * W
KW = 7
NK = 3  # contraction chunks of 128
NM = 3  # output-channel chunks of 128
NTILE = 512
N_CORES = 8
SB = B // N_CORES  # batches per core
G = 4  # pipeline groups per core
BG = SB // G  # batches per group
FREE = BG * HW  # free dim per tile
LEAD = 3  # read-base offset = d of the first (largest-d) group
PADLEN = C * FREE + 2 * LEAD  # + one pad elem between the 7 groups

_CACHE = {}

# residues ordered by descending shift d = (r+3)%7-3
_RES_ORDER = sorted(range(KW), key=lambda r: -((r + 3) % KW - KW // 2))


def _perm_and_segs():
    """Channel permutation (residue groups, descending d) and per-chunk
    segments segs[k] = [(p0, p1, d)] (partitions [128k+p0, 128k+p1))."""
    perm = []
    segs = [[] for _ in range(NK)]
    i = 0
    for r in _RES_ORDER:
        chans = [c for c in range(C) if c % KW == r]
        perm.extend(chans)
        d = (r + 3) % KW - KW // 2
        lo, hi = i, i + len(chans)
        while lo < hi:
            k = lo // 128
            seg_hi = min(hi, (k + 1) * 128)
            segs[k].append((lo - 128 * k, seg_hi - 128 * k, d))
            lo = seg_hi
        i = hi
    return np.asarray(perm), segs


_PERM, _SEGS = _perm_and_segs()
# group boundaries in permuted index space (for pad insertion)
_GROUP_SIZES = [len([c for c in range(C) if c % KW == r]) for r in _RES_ORDER]


def _prep_weights(weight, bias):
    import ml_dtypes

    # weights scaled to +-2: the device computes g = (x >= 0) - 0.5 in {+-0.5},
    # so products are +-1 and PSUM sums match the +-1 x +-1 reference exactly
    wb = np.sign(weight.astype(np.float32)) * 2.0  # [O, C]
    lhsT = np.ascontiguousarray(wb.T[_PERM, :])  # [C_perm, O]
    wt = np.ascontiguousarray(lhsT.reshape(NK, 128, C).transpose(1, 0, 2)).astype(
        ml_dtypes.float8_e5m2
    )  # [128, NK, C]
    bias_sb = np.ascontiguousarray(bias.astype(np.float32).reshape(NM, 128).T)

    mask = np.ones((128, NK, W), dtype=np.float32)
    for k in range(NK):
        for (p0, p1, d) in _SEGS[k]:
            if d > 0:
                mask[p0:p1, k, W - d : W] = 0.0
            elif d < 0:
                mask[p0:p1, k, 0:-d] = 0.0
    mask = mask.astype(ml_dtypes.float8_e5m2)
    return wt, bias_sb, mask


def _legalize_waits(nc, max_waits=1):
    """Walrus for this toolchain accepts at most one sem wait per
    instruction.  Split instructions carrying more into preceding
    same-engine NoOps (engine streams are in-order, so the split is
    semantically identical to the combined wait)."""
    import concourse.mybir as mybir

    fn = nc.m.functions[0]
    ctr = 0
    for blk in fn.blocks:
        out = []
        changed = False
        for inst in blk.instructions:
            si = inst.sync_info
            waits = list(si.on_wait) if si is not None and si.on_wait else []
            if len(waits) > max_waits and str(inst.engine) != "EngineType.Unassigned":
                keep = waits[-max_waits:]
                extra = waits[:-max_waits]
                for j in range(0, len(extra), max_waits):
                    nop = mybir.InstNoOp(name=f"I-waitsplit-{ctr}")
                    ctr += 1
                    nop.engine = inst.engine
                    nop.sync_info = mybir.SyncInfo(
                        on_wait=extra[j : j + max_waits], on_update=[]
                    )
                    out.append(nop)
                si.on_wait = keep
                changed = True
            out.append(inst)
        if changed:
            blk.instructions = out
    return ctr


def _build(psum_bufs=8, ost_bufs=4, legalize=True):
    import concourse.bass as bass
    import concourse.mybir as mybir
    import concourse.tile as tile
    from concourse.ap import AP

    nc = bass.Bass()
    x_d = nc.declare_dram_parameter("x", [G, PADLEN], mybir.dt.float8e5, isOutput=False)
    wt_d = nc.declare_dram_parameter("wt", [128, NK, C], mybir.dt.float8e5, isOutput=False)
    bias_d = nc.declare_dram_parameter("bias", [128, NM], mybir.dt.float32, isOutput=False)
    mask_d = nc.declare_dram_parameter("mask", [128, NK, W], mybir.dt.float8e5, isOutput=False)
    out_d = nc.declare_dram_parameter("out", [G, C, FREE], mybir.dt.bfloat16, isOutput=True)

    with tile.TileContext(nc) as tc:
        with (
            tc.tile_pool(name="const", bufs=1) as const_pool,
            tc.tile_pool(name="raw", bufs=G * NK) as raw_pool,
            tc.tile_pool(name="g", bufs=G) as g_pool,
            tc.tile_pool(name="ost", bufs=ost_bufs) as ost_pool,
            tc.tile_pool(name="ps", bufs=psum_bufs, space="PSUM") as ps_pool,
        ):
            wt = const_pool.tile([128, NK, C], mybir.dt.float8e5)
            bias_sb = const_pool.tile([128, NM], mybir.dt.float32)
            mask_sb = const_pool.tile([128, NK, W], mybir.dt.float8e5)
            nc.sync.dma_start(wt[:], wt_d[:])
            nc.sync.dma_start(bias_sb[:], bias_d[:])
            nc.sync.dma_start(mask_sb[:], mask_d[:])

            # phase L: all input loads up front (dense full-partition 2D)
            raws = {}
            for g in range(G):
                for k in range(NK):
                    raw = raw_pool.tile([128, FREE], mybir.dt.float8e5, tag="raw")
                    src = AP(
                        tensor=x_d,
                        offset=g * PADLEN + LEAD + k * 128 * FREE,
                        ap=[[FREE, 128], [1, FREE]],
                    )
                    nc.sync.dma_start(raw[:], src)
                    raws[(g, k)] = raw

            # phase C: binarize g = (x >= 0) - 0.5 in {+-0.5} (one fused
            # two-op tensor_scalar on DVE) + boundary mask multiply (DVE)
            g01s = {}
            g2s = {}
            for g in range(G):
                g01 = g_pool.tile([128, 2, FREE], mybir.dt.float8e5, tag="g01")
                g2 = g_pool.tile([128, FREE], mybir.dt.float8e5, tag="g2")
                for k in range(NK):
                    raw = raws[(g, k)]
                    gk = g01[:, k, :] if k < 2 else g2[:]
                    nc.vector.tensor_scalar(
                        gk,
                        raw[:],
                        0.0,
                        0.5,
                        mybir.AluOpType.is_ge,
                        mybir.AluOpType.subtract,
                    )
                    if k < 2:
                        v = g01.rearrange("p two (r w) -> p two r w", w=W)[:, k]
                    else:
                        v = g2.rearrange("p (r w) -> p r w", w=W)
                    rmax = max([d for _, _, d in _SEGS[k]] + [0])
                    lmax = max([-d for _, _, d in _SEGS[k]] + [0])
                    if rmax:
                        nc.vector.tensor_mul(
                            v[:, :, W - rmax : W],
                            v[:, :, W - rmax : W],
                            mask_sb[:, k : k + 1, W - rmax : W].broadcast_to(
                                [128, BG * H, rmax]
                            ),
                        )
                    if lmax:
                        nc.vector.tensor_mul(
                            v[:, :, 0:lmax],
                            v[:, :, 0:lmax],
                            mask_sb[:, k : k + 1, 0:lmax].broadcast_to(
                                [128, BG * H, lmax]
                            ),
                        )
                g01s[g] = g01
                g2s[g] = g2

            # phase M: GEMM (k-outer so consecutive matmuls share weights) +
            # bias eviction on ScalarE + stores (Sync ring, emitted after
            # every load so they never block a load issue)
            NT = FREE // NTILE
            for g in range(G):
                for m in range(NM):
                    pss = []
                    for _n in range(NT):
                        ps = ps_pool.tile(
                            [128, NTILE], mybir.dt.float32, tag="ps", name=f"ps{_n}"
                        )
                        pss.append(ps)
                    for n in range(NT):
                        nc.tensor.matmul(
                            pss[n][:],
                            wt[:, 0:2, m * 128 : (m + 1) * 128],
                            g01s[g][:, :, n * NTILE : (n + 1) * NTILE],
                            start=True,
                            stop=False,
                            perf_mode=mybir.MatmulPerfMode.DoubleRow,
                        )
                    for n in range(NT):
                        nc.tensor.matmul(
                            pss[n][:],
                            wt[:, 2, m * 128 : (m + 1) * 128],
                            g2s[g][:, n * NTILE : (n + 1) * NTILE],
                            start=False,
                            stop=True,
                        )
                    ost = ost_pool.tile([128, FREE], mybir.dt.bfloat16, tag="ost")
                    for n in range(NT):
                        nc.scalar.activation(
                            ost[:, n * NTILE : (n + 1) * NTILE],
                            pss[n][:],
                            mybir.ActivationFunctionType.Identity,
                            bias=bias_sb[:, m : m + 1],
                        )
                    nc.sync.dma_start(
                        out_d[g, m * 128 : (m + 1) * 128, :], ost[:]
                    )
    if legalize:
        _legalize_waits(nc)
    return nc


def _ensure_ntff_hook():
    """Register the axon NTFF profiling hook if the image's antenv lacks it."""
    import types

    try:
        from antenv.axon_hooks import get_axon_ntff_profile_hook  # noqa: F401

        return
    except ImportError:
        pass
    hook = None
    try:
        from trn_agent_boot.trn_boot import _ntff_profile_via_ctypes

        hook = _ntff_profile_via_ctypes("/opt/axon/libaxon_pjrt.so")
    except Exception:
        pass
    mod = types.ModuleType("antenv.axon_hooks")
    mod._hook = hook
    mod.get_axon_ntff_profile_hook = lambda: mod._hook
    mod.set_axon_ntff_profile_hook = lambda h: setattr(mod, "_hook", h)
    sys.modules["antenv.axon_hooks"] = mod
    try:
        import antenv

        antenv.axon_hooks = mod
    except Exception:
        pass


def _pack_x(x):
    """[B, C, H, W] f32 -> per-core [G, PADLEN] bf16 buffers with the
    permuted channel-major layout and 1-elem pads between shift groups."""
    import ml_dtypes

    xq = np.asarray(x, dtype=np.float32).reshape(B, C, HW)[:, _PERM, :].astype(
        ml_dtypes.float8_e5m2
    )
    shards = []
    for i in range(N_CORES):
        buf = np.zeros((G, PADLEN), dtype=ml_dtypes.float8_e5m2)
        for g in range(G):
            src = xq[i * SB + g * BG : i * SB + (g + 1) * BG]  # [BG, C, HW]
            xt = np.ascontiguousarray(src.transpose(1, 0, 2)).reshape(C, FREE)
            pos = 0
            a = 0
            for n in _GROUP_SIZES:
                buf[g, pos : pos + n * FREE] = xt[a : a + n].reshape(-1)
                pos += n * FREE + 1
                a += n
        shards.append(buf)
    return shards


def run(x, weight, bias, trace=False):
    """Returns (out [B,C,H,W] f32, exec_time_ns or None)."""
    import concourse.bass_utils as bu
    from concourse.bass_utils import run_bass_kernel_spmd

    if trace:
        _ensure_ntff_hook()
        # zero-egress container: don't try to copy trace artifacts to a bucket
        bu.upload_artifacts = lambda tmpdir: tmpdir

    if "nc" not in _CACHE:
        _CACHE["nc"] = _build()
    nc = _CACHE["nc"]

    wt, bias_sb, mask = _prep_weights(weight, bias)
    shards = _pack_x(x)
    in_maps = [
        {"x": shards[i], "wt": wt, "bias": bias_sb, "mask": mask}
        for i in range(N_CORES)
    ]
    res = run_bass_kernel_spmd(
        nc, in_maps, core_ids=list(range(N_CORES)), trace=trace
    )
    outs = []
    for i in range(N_CORES):
        o = np.asarray(res.results[i]["out"]).reshape(G, C, BG, HW)
        outs.append(o.transpose(0, 2, 1, 3).reshape(SB, C, HW))
    out = np.concatenate(outs, axis=0).astype(np.float32)
    return out.reshape(B, C, H, W), res.exec_time_ns


def kernel(x, weight, bias):
    out, _ = run(x, weight, bias, trace=False)
    return out


# revision 19
# speedup vs baseline: 1.2366x; 1.0014x over previous
"""CycleMLP 1w1a (binary cycle-shift conv + 1x1 GEMM) for 8 Trainium2 cores.

  out[b,o,h,w] = sum_c sign(weight)[o,c] * sign(x)[b,c,h,w+off(c)] + bias[o]
  off(c) = (c+3) % 7 - 3, zero-padded outside [0, W)

Sharding: data-parallel over batch B=64 -> 8 batches/core; weight/bias
replicated.

Key layout tricks (all host-side, mathematically identity):
  - channels permuted so shift-groups (residue c % 7) are contiguous and
    ordered by DESCENDING shift d; the weight's contraction dim is permuted
    identically.  With one pad element inserted between groups in the DRAM
    buffer, consecutive shifted per-channel windows tile the buffer exactly
    contiguously, so each 128-channel chunk loads as ONE dense full-partition
    2D DMA (the only transfer shape that fans evenly across all 16 SDMA
    engines; sub-128-partition or lattice transfers pile onto engine 0).
    The shift itself is still performed by the device DMA via the base
    offset; all group/row-boundary leaks land in masked columns.
  - per-core layout [G, C*BG*HW] (channel-major per half-batch group):
    6 input loads + 6 stores of ~1 MB each per core, total.
  - fp8e5m2 x (sign-preserving except |x| < 2^-17, P ~ 3e-6) and bf16 out
    (integer sums <= 384 round with ulp 0.125 -> rel err ~5e-4) cut HBM
    traffic to 0.25x + 0.5x of f32.

Per-core kernel, phase-ordered so no engine stream blocks another:
  all loads (Sync HWDGE) | sign per chunk (ScalarE) | boundary mask-mul
  (GpSimd) | GEMM 3m x 8n x 3k with PSUM K-accum (TensorE) | bias-add
  eviction to bf16 (DVE) | stores (Sync HWDGE, queued after all loads).
"""

import sys

for p in ("/opt/trn_rl_repo", "/root/.axon_site/_ro/trn_rl_repo"):
    if p not in sys.path:
        sys.path.append(p)

import numpy as np

B = 64
C = 384
H = W = 32
HW = H * W
KW = 7
NK = 3  # contraction chunks of 128
NM = 3  # output-channel chunks of 128
NTILE = 512
N_CORES = 8
SB = B // N_CORES  # batches per core
G = 4  # pipeline groups per core
BG = SB // G  # batches per group
FREE = BG * HW  # free dim per tile
LEAD = 3  # read-base offset = d of the first (largest-d) group
PADLEN = C * FREE + 2 * LEAD  # + one pad elem between the 7 groups

_CACHE = {}

# residues ordered by descending shift d = (r+3)%7-3
_RES_ORDER = sorted(range(KW), key=lambda r: -((r + 3) % KW - KW // 2))


def _perm_and_segs():
    """Channel permutation (residue groups, descending d) and per-chunk
    segments segs[k] = [(p0, p1, d)] (partitions [128k+p0, 128k+p1))."""
    perm = []
    segs = [[] for _ in range(NK)]
    i = 0
    for r in _RES_ORDER:
        chans = [c for c in range(C) if c % KW == r]
        perm.extend(chans)
        d = (r + 3) % KW - KW // 2
        lo, hi = i, i + len(chans)
        while lo < hi:
            k = lo // 128
            seg_hi = min(hi, (k + 1) * 128)
            segs[k].append((lo - 128 * k, seg_hi - 128 * k, d))
            lo = seg_hi
        i = hi
    return np.asarray(perm), segs


_PERM, _SEGS = _perm_and_segs()
# group boundaries in permuted index space (for pad insertion)
_GROUP_SIZES = [len([c for c in range(C) if c % KW == r]) for r in _RES_ORDER]


def _prep_weights(weight, bias):
    import ml_dtypes

    # weights scaled to +-2: the device computes g = (x >= 0) - 0.5 in {+-0.5},
    # so products are +-1 and PSUM sums match the +-1 x +-1 reference exactly
    wb = np.sign(weight.astype(np.float32)) * 2.0  # [O, C]
    lhsT = np.ascontiguousarray(wb.T[_PERM, :])  # [C_perm, O]
    wt = np.ascontiguousarray(lhsT.reshape(NK, 128, C).transpose(1, 0, 2)).astype(
        ml_dtypes.float8_e5m2
    )  # [128, NK, C]
    bias_sb = np.ascontiguousarray(bias.astype(np.float32).reshape(NM, 128).T)

    mask = np.ones((128, NK, W), dtype=np.float32)
    for k in range(NK):
        for (p0, p1, d) in _SEGS[k]:
            if d > 0:
                mask[p0:p1, k, W - d : W] = 0.0
            elif d < 0:
                mask[p0:p1, k, 0:-d] = 0.0
    mask = mask.astype(ml_dtypes.float8_e5m2)
    return wt, bias_sb, mask


def _legalize_waits(nc, max_waits=1):
    """Walrus for this toolchain accepts at most one sem wait per
    instruction.  Split instructions carrying more into preceding
    same-engine NoOps (engine streams are in-order, so the split is
    semantically identical to the combined wait)."""
    import concourse.mybir as mybir

    fn = nc.m.functions[0]
    ctr = 0
    for blk in fn.blocks:
        out = []
        changed = False
        for inst in blk.instructions:
            si = inst.sync_info
            waits = list(si.on_wait) if si is not None and si.on_wait else []
            if len(waits) > max_waits and str(inst.engine) != "EngineType.Unassigned":
                keep = waits[-max_waits:]
                extra = waits[:-max_waits]
                for j in range(0, len(extra), max_waits):
                    nop = mybir.InstNoOp(name=f"I-waitsplit-{ctr}")
                    ctr += 1
                    nop.engine = inst.engine
                    nop.sync_info = mybir.SyncInfo(
                        on_wait=extra[j : j + max_waits], on_update=[]
                    )
                    out.append(nop)
                si.on_wait = keep
                changed = True
            out.append(inst)
        if changed:
            blk.instructions = out
    return ctr


def _build(psum_bufs=8, ost_bufs=4, legalize=True):
    import concourse.bass as bass
    import concourse.mybir as mybir
    import concourse.tile as tile
    from concourse.ap import AP

    nc = bass.Bass()
    x_d = nc.declare_dram_parameter("x", [G, PADLEN], mybir.dt.float8e5, isOutput=False)
    wt_d = nc.declare_dram_parameter("wt", [128, NK, C], mybir.dt.float8e5, isOutput=False)
    bias_d = nc.declare_dram_parameter("bias", [128, NM], mybir.dt.float32, isOutput=False)
    mask_d = nc.declare_dram_parameter("mask", [128, NK, W], mybir.dt.float8e5, isOutput=False)
    out_d = nc.declare_dram_parameter("out", [G, C, FREE], mybir.dt.bfloat16, isOutput=True)

    with tile.TileContext(nc) as tc:
        with (
            tc.tile_pool(name="const", bufs=1) as const_pool,
            tc.tile_pool(name="raw", bufs=G * NK) as raw_pool,
            tc.tile_pool(name="g", bufs=G) as g_pool,
            tc.tile_pool(name="ost", bufs=ost_bufs) as ost_pool,
            tc.tile_pool(name="ps", bufs=psum_bufs, space="PSUM") as ps_pool,
        ):
            wt = const_pool.tile([128, NK, C], mybir.dt.float8e5)
            bias_sb = const_pool.tile([128, NM], mybir.dt.float32)
            mask_sb = const_pool.tile([128, NK, W], mybir.dt.float8e5)
            nc.sync.dma_start(wt[:], wt_d[:])
            nc.sync.dma_start(bias_sb[:], bias_d[:])
            nc.sync.dma_start(mask_sb[:], mask_d[:])

            # phase L: all input loads up front (dense full-partition 2D)
            raws = {}
            for g in range(G):
                for k in range(NK):
                    raw = raw_pool.tile([128, FREE], mybir.dt.float8e5, tag="raw")
                    src = AP(
                        tensor=x_d,
                        offset=g * PADLEN + LEAD + k * 128 * FREE,
                        ap=[[FREE, 128], [1, FREE]],
                    )
                    nc.sync.dma_start(raw[:], src)
                    raws[(g, k)] = raw

            # phase C: binarize g = (x >= 0) - 0.5 in {+-0.5} (one fused
            # two-op tensor_scalar on DVE) + boundary mask multiply (DVE)
            g01s = {}
            g2s = {}
            for g in range(G):
                g01 = g_pool.tile([128, 2, FREE], mybir.dt.float8e5, tag="g01")
                g2 = g_pool.tile([128, FREE], mybir.dt.float8e5, tag="g2")
                for k in range(NK):
                    raw = raws[(g, k)]
                    gk = g01[:, k, :] if k < 2 else g2[:]
                    nc.vector.tensor_scalar(
                        gk,
                        raw[:],
                        0.0,
                        0.5,
                        mybir.AluOpType.is_ge,
                        mybir.AluOpType.subtract,
                    )
                    if k < 2:
                        v = g01.rearrange("p two (r w) -> p two r w", w=W)[:, k]
                    else:
                        v = g2.rearrange("p (r w) -> p r w", w=W)
                    rmax = max([d for _, _, d in _SEGS[k]] + [0])
                    lmax = max([-d for _, _, d in _SEGS[k]] + [0])
                    if rmax:
                        nc.vector.tensor_mul(
                            v[:, :, W - rmax : W],
                            v[:, :, W - rmax : W],
                            mask_sb[:, k : k + 1, W - rmax : W].broadcast_to(
                                [128, BG * H, rmax]
                            ),
                        )
                    if lmax:
                        nc.vector.tensor_mul(
                            v[:, :, 0:lmax],
                            v[:, :, 0:lmax],
                            mask_sb[:, k : k + 1, 0:lmax].broadcast_to(
                                [128, BG * H, lmax]
                            ),
                        )
                g01s[g] = g01
                g2s[g] = g2

            # phase M: GEMM (k-outer so consecutive matmuls share weights) +
            # bias eviction on ScalarE + stores (Sync ring, emitted after
            # every load so they never block a load issue)
            NT = FREE // NTILE
            for g in range(G):
                for m in range(NM):
                    pss = []
                    for _n in range(NT):
                        ps = ps_pool.tile(
                            [128, NTILE], mybir.dt.float32, tag="ps", name=f"ps{_n}"
                        )
                        pss.append(ps)
                    for n in range(NT):
                        nc.tensor.matmul(
                            pss[n][:],
                            wt[:, 0:2, m * 128 : (m + 1) * 128],
                            g01s[g][:, :, n * NTILE : (n + 1) * NTILE],
                            start=True,
                            stop=False,
                            perf_mode=mybir.MatmulPerfMode.DoubleRow,
                        )
                    for n in range(NT):
                        nc.tensor.matmul(
                            pss[n][:],
                            wt[:, 2, m * 128 : (m + 1) * 128],
                            g2s[g][:, n * NTILE : (n + 1) * NTILE],
                            start=False,
                            stop=True,
                        )
                    ost = ost_pool.tile([128, FREE], mybir.dt.bfloat16, tag="ost")
                    for n in range(NT):
                        # alternate eviction between ScalarE and DVE so the
                        # drain tracks the matmuls instead of trailing them
                        if n % 2 == 0:
                            nc.scalar.activation(
                                ost[:, n * NTILE : (n + 1) * NTILE],
                                pss[n][:],
                                mybir.ActivationFunctionType.Identity,
                                bias=bias_sb[:, m : m + 1],
                            )
                        else:
                            nc.vector.tensor_scalar_add(
                                ost[:, n * NTILE : (n + 1) * NTILE],
                                pss[n][:],
                                bias_sb[:, m : m + 1],
                            )
                    nc.sync.dma_start(
                        out_d[g, m * 128 : (m + 1) * 128, :], ost[:]
                    )
    if legalize:
        _legalize_waits(nc)
    return nc


def _ensure_ntff_hook():
    """Register the axon NTFF profiling hook if the image's antenv lacks it."""
    import types

    try:
        from antenv.axon_hooks import get_axon_ntff_profile_hook  # noqa: F401

        return
    except ImportError:
        pass
    hook = None
    try:
        from trn_agent_boot.trn_boot import _ntff_profile_via_ctypes

        hook = _ntff_profile_via_ctypes("/opt/axon/libaxon_pjrt.so")
    except Exception:
        pass
    mod = types.ModuleType("antenv.axon_hooks")
    mod._hook = hook
    mod.get_axon_ntff_profile_hook = lambda: mod._hook
    mod.set_axon_ntff_profile_hook = lambda h: setattr(mod, "_hook", h)
    sys.modules["antenv.axon_hooks"] = mod
    try:
        import antenv

        antenv.axon_hooks = mod
    except Exception:
        pass


def _pack_x(x):
    """[B, C, H, W] f32 -> per-core [G, PADLEN] bf16 buffers with the
    permuted channel-major layout and 1-elem pads between shift groups."""
    import ml_dtypes

    xq = np.asarray(x, dtype=np.float32).reshape(B, C, HW)[:, _PERM, :].astype(
        ml_dtypes.float8_e5m2
    )
    shards = []
    for i in range(N_CORES):
        buf = np.zeros((G, PADLEN), dtype=ml_dtypes.float8_e5m2)
        for g in range(G):
            src = xq[i * SB + g * BG : i * SB + (g + 1) * BG]  # [BG, C, HW]
            xt = np.ascontiguousarray(src.transpose(1, 0, 2)).reshape(C, FREE)
            pos = 0
            a = 0
            for n in _GROUP_SIZES:
                buf[g, pos : pos + n * FREE] = xt[a : a + n].reshape(-1)
                pos += n * FREE + 1
                a += n
        shards.append(buf)
    return shards


def run(x, weight, bias, trace=False):
    """Returns (out [B,C,H,W] f32, exec_time_ns or None)."""
    import concourse.bass_utils as bu
    from concourse.bass_utils import run_bass_kernel_spmd

    if trace:
        _ensure_ntff_hook()
        # zero-egress container: don't try to copy trace artifacts to a bucket
        bu.upload_artifacts = lambda tmpdir: tmpdir

    if "nc" not in _CACHE:
        _CACHE["nc"] = _build()
    nc = _CACHE["nc"]

    wt, bias_sb, mask = _prep_weights(weight, bias)
    shards = _pack_x(x)
    in_maps = [
        {"x": shards[i], "wt": wt, "bias": bias_sb, "mask": mask}
        for i in range(N_CORES)
    ]
    res = run_bass_kernel_spmd(
        nc, in_maps, core_ids=list(range(N_CORES)), trace=trace
    )
    outs = []
    for i in range(N_CORES):
        o = np.asarray(res.results[i]["out"]).reshape(G, C, BG, HW)
        outs.append(o.transpose(0, 2, 1, 3).reshape(SB, C, HW))
    out = np.concatenate(outs, axis=0).astype(np.float32)
    return out.reshape(B, C, H, W), res.exec_time_ns


def kernel(x, weight, bias):
    out, _ = run(x, weight, bias, trace=False)
    return out
